# revision 6
# baseline (speedup 1.0000x reference)
"""AugmentedTripletLoss on 8 TRN2 NeuronCores — data-parallel Bass kernel.

v3 design: two collective-free NEFF passes + tiny host reductions.

The v2 baseline's NEFF span was dominated by its three AllReduce
collectives: under staggered multi-core launch every core's span absorbs
the full launch skew at the first collective barrier, and the collective
itself is slow through the runtime.  All cross-core reduction here is
[16, 513] / [16, 17] — small enough to gather on the host instead.

  Pass 1 (per core, no collectives): stream the core's 16384 bf16
    embeddings (4-sample-packed rows, 4KB DMA lines), accumulate
    one-hot^T @ emb class sums into a single [16, 512] PSUM bank
    (128 matmuls), DMA the [16, 512] partial out.  One-hot masks are
    precomputed on the host and shipped as a [128, 2048] bf16 input.
  Host: reduce the 8 class-sum partials, bincount labels, centroids,
    normalized chat, close-pair mask pm, deg  (all [16, x] numpy).
  Pass 2 (per core, no collectives): stream the core's normalized
    embeddings in [d, s] layout (8 blocks of [128, 4x2048]), per
    128-sample tile compute cos = ehat . chat via 4 PSUM matmuls,
    two Relu activations (inter / intra terms), masked row-sum via
    one fused DVE op, and accumulate S^T[16,16] ++ t[16,1] in PSUM
    via one-hot matmuls.  DMA the [16, 17] partial out.
  Host: reduce the 8 [16,17] partials, assemble the scalar loss.
"""

import sys

sys.path.insert(0, "/opt/trn_rl_repo")

import numpy as np

import concourse.bass as bass
import concourse.bacc as bacc
import concourse.tile as tile
import concourse.mybir as mybir
from concourse.bass_utils import run_bass_kernel_spmd

ALPHA = 0.1
BETA = 1.1
EPS = 1e-8
C = 16
N = 131072
D = 512
CORES = 8
NL = N // CORES  # 16384 samples per core
P = 128
T = NL // P      # 128 tiles per core
KCH = D // P     # 4 contraction chunks of 128
W = 2048         # pass-2 streaming block width (samples)
NBLK = NL // W   # 8 blocks
TPB = W // P     # 16 tiles per block

F32 = mybir.dt.float32
BF16 = mybir.dt.bfloat16
ALU = mybir.AluOpType
ACTF = mybir.ActivationFunctionType

_CACHE = {}


def _build_p1():
    """Per-core class sums: out1[16, 512] = sum_t onehot_t^T @ emb_t."""
    nc = bacc.Bacc("TRN2", target_bir_lowering=False, debug=False,
                   num_devices=CORES)
    emb = nc.dram_tensor("emb", [NL // 4, 4 * D], BF16, kind="ExternalInput")
    oh_in = nc.dram_tensor("oh", [P, T * C], BF16, kind="ExternalInput")
    out = nc.dram_tensor("out", [C, D], F32, kind="ExternalOutput")

    with tile.TileContext(nc) as tc:
        with (
            tc.tile_pool(name="pers", bufs=1) as pers,
            tc.tile_pool(name="ld", bufs=4) as ld,
            tc.tile_pool(name="small", bufs=1) as small,
            tc.tile_pool(name="ps", bufs=1, space="PSUM") as ps,
        ):
            oh = pers.tile([P, T * C], BF16)
            nc.sync.dma_start(oh[:], oh_in[:, :])
            ps_sums = ps.tile([C, D], F32)
            for g in range(T // 4):
                ebf = ld.tile([P, 4 * D], BF16)
                nc.sync.dma_start(ebf[:], emb[g * P:(g + 1) * P, :])
                for h in range(4):
                    t = 4 * g + h
                    nc.tensor.matmul(ps_sums[:], oh[:, t * C:(t + 1) * C],
                                     ebf[:, h * D:(h + 1) * D],
                                     start=(t == 0), stop=(t == T - 1))
            loc = small.tile([C, D], F32)
            nc.vector.tensor_copy(loc[:], ps_sums[:])
            nc.sync.dma_start(out.ap()[:, :], loc[:])
    nc.compile()
    return nc


def _build_p2():
    """Per-core S^T[16,16] ++ intra t[16,1] partials: out2[16, 17]."""
    nc = bacc.Bacc("TRN2", target_bir_lowering=False, debug=False,
                   num_devices=CORES)
    embT = nc.dram_tensor("embT", [D, NL], BF16, kind="ExternalInput")
    oh_in = nc.dram_tensor("oh", [P, T * C], BF16, kind="ExternalInput")
    chT_in = nc.dram_tensor("chT", [P, KCH * C], BF16, kind="ExternalInput")
    out = nc.dram_tensor("out", [C, C + 1], F32, kind="ExternalOutput")

    with tile.TileContext(nc) as tc:
        with (
            tc.tile_pool(name="pers", bufs=1) as pers,
            tc.tile_pool(name="ld", bufs=3) as ld,
            tc.tile_pool(name="work", bufs=4) as work,
            tc.tile_pool(name="small", bufs=1) as small,
            tc.tile_pool(name="psacc", bufs=1, space="PSUM") as psacc,
            tc.tile_pool(name="pstr", bufs=2, space="PSUM") as pstr,
        ):
            oh = pers.tile([P, T * C], BF16)
            nc.sync.dma_start(oh[:], oh_in[:, :])
            chT = pers.tile([P, KCH * C], BF16)
            nc.sync.dma_start(chT[:], chT_in[:, :])
            bq = pers.tile([P, 1], F32)
            nc.vector.memset(bq[:], float(BETA - 1.0))
            br = pers.tile([P, 1], F32)
            nc.vector.memset(br[:], float(1.0 - ALPHA))

            ps_st = psacc.tile([C, C + 1], F32)
            for j in range(NBLK):
                eTb = ld.tile([P, KCH * W], BF16)
                for k in range(KCH):
                    nc.sync.dma_start(
                        eTb[:, k * W:(k + 1) * W],
                        embT[k * P:(k + 1) * P, j * W:(j + 1) * W])
                for tt in range(TPB):
                    t = j * TPB + tt
                    dot = pstr.tile([P, C], F32, tag="dot")
                    for k in range(KCH):
                        nc.tensor.matmul(
                            dot[:], eTb[:, k * W + tt * P:k * W + (tt + 1) * P],
                            chT[:, k * C:(k + 1) * C],
                            start=(k == 0), stop=(k == KCH - 1))
                    qr = work.tile([P, C + 1], BF16)
                    # inter: relu(cos + (BETA-1)); intra: relu(-cos + (1-ALPHA))
                    nc.scalar.activation(qr[:, :C], dot[:], ACTF.Relu,
                                         bias=bq[:], scale=1.0)
                    rt = work.tile([P, C], F32)
                    nc.scalar.activation(rt[:], dot[:], ACTF.Relu,
                                         bias=br[:], scale=-1.0)
                    rr = work.tile([P, C], F32)
                    rsum = work.tile([P, 1], F32)
                    nc.vector.scalar_tensor_tensor(
                        rr[:], rt[:], 1.0, oh[:, t * C:(t + 1) * C],
                        ALU.mult, ALU.mult, accum_out=rsum[:])
                    nc.vector.tensor_copy(qr[:, C:C + 1], rsum[:])
                    nc.tensor.matmul(ps_st[:], oh[:, t * C:(t + 1) * C], qr[:],
                                     start=(t == 0), stop=(t == T - 1))
            loc = small.tile([C, C + 1], F32)
            nc.vector.tensor_copy(loc[:], ps_st[:])
            nc.sync.dma_start(out.ap()[:, :], loc[:])
    nc.compile()
    return nc


def _host_prep(embeddings, labels):
    import ml_dtypes
    embf = np.asarray(embeddings, dtype=np.float32)
    emb_bf = embf.astype(ml_dtypes.bfloat16)
    nrm = np.maximum(np.sqrt((embf * embf).sum(1, keepdims=True)), EPS)
    ehat = (embf / nrm).astype(ml_dtypes.bfloat16)
    lab = np.asarray(labels).astype(np.int64)
    onehot = (lab[:, None] == np.arange(C)[None, :])

    in1, in2, oh_shards = [], [], []
    for i in range(CORES):
        sl = slice(i * NL, (i + 1) * NL)
        esh = np.ascontiguousarray(
            emb_bf[sl].reshape(T // 4, 4, P, D)
            .transpose(0, 2, 1, 3).reshape(NL // 4, 4 * D))
        esT = np.ascontiguousarray(ehat[sl].T)
        ohs = np.ascontiguousarray(
            onehot[sl].reshape(T, P, C).transpose(1, 0, 2)
            .reshape(P, T * C).astype(ml_dtypes.bfloat16))
        in1.append({"emb": esh, "oh": ohs})
        in2.append({"embT": esT, "oh": ohs})
    return in1, in2, lab


def kernel(embeddings: np.ndarray, labels: np.ndarray) -> np.ndarray:
    import ml_dtypes
    if "p1" not in _CACHE:
        _CACHE["p1"] = _build_p1()
        _CACHE["p2"] = _build_p2()
    nc1, nc2 = _CACHE["p1"], _CACHE["p2"]

    in1, in2, lab = _host_prep(embeddings, labels)

    r1 = run_bass_kernel_spmd(nc1, in1, core_ids=list(range(CORES)))
    sums = np.zeros((C, D), np.float64)
    for res in r1.results:
        sums += res["out"].astype(np.float64)

    cnt = np.bincount(lab, minlength=C).astype(np.float64)
    cent = sums / np.maximum(cnt, 1.0)[:, None]
    present = cnt > 0
    cnorm = np.maximum(np.sqrt((cent * cent).sum(1)), EPS)
    chat = cent / cnorm[:, None]
    pd = 1.0 - chat @ chat.T
    upper = np.triu(np.ones((C, C), bool), k=1)
    pairmask = upper & (pd <= BETA) & present[:, None] & present[None, :]
    pm = pairmask.astype(np.float64)
    deg = pm.sum(1) + pm.sum(0)
    num_pairs = pm.sum()

    chat_bf = chat.astype(np.float32).astype(ml_dtypes.bfloat16)
    chT = np.zeros((P, KCH * C), ml_dtypes.bfloat16)
    for k in range(KCH):
        chT[:, k * C:(k + 1) * C] = chat_bf[:, k * P:(k + 1) * P].T
    for m in in2:
        m["chT"] = chT

    r2 = run_bass_kernel_spmd(nc2, in2, core_ids=list(range(CORES)))
    st = np.zeros((C, C + 1), np.float64)
    for res in r2.results:
        st += res["out"].astype(np.float64)
    S = st[:, :C].T
    tvec = st[:, C]

    inter_sum = (pm * (S + S.T)).sum()
    intra_sum = (deg * tvec).sum()
    count = (deg * cnt).sum()
    loss = (intra_sum + inter_sum) / max(count, 1.0) if num_pairs > 0 else 0.0
    return np.float32(loss)
